# revision 1
# baseline (speedup 1.0000x reference)
"""MatchingNetwork forward on 8 Trainium2 NeuronCores.

The reference network's output reduces exactly to one_hot(labels, V) in f32:
the final einsum('btn,btv->btv', att, one_hot) sums att over n, and att is a
softmax over n, so the output is one_hot scaled by sum(softmax) == 1 (to float
rounding, ~1e-7).  Everything upstream (embedding gathers, BiLSTM GLayer,
attentional FLayer) cancels out of the result for every input.

So the kernel is a distributed one-hot materialization: B*T = 2048 rows of
V = 32000 f32 each (262 MB of output), data-parallel over rows across 8 cores
(256 rows/core = 32.77 MB/core of pure HBM writes -> memory-bound).

Per core: labels arrive as [128 partitions, 2] (row r = b*128 + p).  A single
SBUF iota row-chunk [128, CHUNK] is compared per (chunk, batch) against the
per-partition label via DVE tensor_scalar(subtract, is_equal), producing
1.0/0.0 f32 tiles that stream straight out to DRAM via HWDGE DMA.
"""

import os
import sys

for _p in ("/opt/trn_rl_repo", "/root/.axon_site/_ro/trn_rl_repo"):
    if os.path.isdir(_p) and _p not in sys.path:
        sys.path.append(_p)

import numpy as np

B, T, V = 32, 64, 32000
N_CORES = 8
ROWS = B * T                 # 2048 one-hot rows total
RPC = ROWS // N_CORES        # 256 rows per core
NB = RPC // 128              # 2 batches of 128 partitions
# Uniform column-chunk width.  2000 measured best: the 1 MB DMAs keep the
# write stream at ~415 GB/s while the first compare only waits on a ~2x1.8us
# gpsimd iota.  (8000 stalls startup ~20us; 1000 and mixed 500..4000 widths
# both measured slower in steady state.)
CHUNK = 2000
NCHUNK = V // CHUNK
MAXCHUNK = CHUNK
# iota built in two serial gpsimd pieces so scheduling can start DMAs while
# the second half generates.
IOTA_PIECES = [(0, 1000), (1000, 2000)]

_cache = {}


def _build_nc():
    import concourse.bacc as bacc
    import concourse.mybir as mybir
    from concourse.tile import TileContext

    nc = bacc.Bacc()
    lab_d = nc.dram_tensor("labels", [128, NB], mybir.dt.float32,
                           kind="ExternalInput")
    out_d = nc.dram_tensor("out", [NB, 128, V], mybir.dt.float32,
                           kind="ExternalOutput")

    with TileContext(nc) as tc:
        with tc.tile_pool(name="const", bufs=1) as cpool, \
             tc.tile_pool(name="work", bufs=6) as wpool:
            lab = cpool.tile([128, NB], mybir.dt.float32, tag="lab")
            nc.sync.dma_start(out=lab[:, :], in_=lab_d[:, :])
            iota = cpool.tile([128, MAXCHUNK], mybir.dt.float32, tag="iota")
            for (s, e) in IOTA_PIECES:
                nc.gpsimd.iota(iota[:, s:e], [[1, e - s]], base=s,
                               channel_multiplier=0,
                               allow_small_or_imprecise_dtypes=True)
            dma_engines = [nc.sync, nc.scalar]
            col = 0
            k = 0
            for w in [CHUNK] * NCHUNK:
                for b in range(NB):
                    o = wpool.tile([128, w], mybir.dt.float32, tag="o")
                    # o = is_equal(iota - (-col), lab[:, b])
                    #   = (global_col == label) ? 1.0 : 0.0
                    nc.vector.tensor_scalar(
                        out=o[:, :], in0=iota[:, :w],
                        scalar1=float(-col), scalar2=lab[:, b:b + 1],
                        op0=mybir.AluOpType.subtract,
                        op1=mybir.AluOpType.is_equal)
                    dma_engines[k % 2].dma_start(
                        out=out_d[b, :, col:col + w], in_=o[:, :])
                    k += 1
                col += w
    nc.finalize()
    return nc


def kernel(**inputs):
    from concourse.bass_utils import run_bass_kernel_spmd

    if "nc" not in _cache:
        _cache["nc"] = _build_nc()
    nc = _cache["nc"]

    # Label values < 2^24 are exact in f32.
    lab = np.asarray(inputs["labels"]).reshape(-1).astype(np.float32)
    in_maps = []
    for i in range(N_CORES):
        shard = lab[i * RPC:(i + 1) * RPC].reshape(NB, 128).T  # [128, NB]
        in_maps.append({"labels": np.ascontiguousarray(shard)})

    trace = bool(int(os.environ.get("BASS_KERNEL_TRACE", "0")))
    res = run_bass_kernel_spmd(nc, in_maps, list(range(N_CORES)), trace=trace)
    _cache["last_res"] = res

    outs = [res.results[i]["out"].reshape(RPC, V) for i in range(N_CORES)]
    return np.concatenate(outs, axis=0).reshape(B, T, V)



# revision 3
# speedup vs baseline: 2.6632x; 2.6632x over previous
"""MatchingNetwork forward on 8 Trainium2 NeuronCores.

The reference network's output reduces exactly to one_hot(labels, V) in f32:
the final einsum('btn,btv->btv', att, one_hot) sums att over n, and att is a
softmax over n, so the output is one_hot scaled by sum(softmax) == 1 (to float
rounding, ~1e-7).  Everything upstream (embedding gathers, BiLSTM GLayer,
attentional FLayer) cancels out of the result for every input.

So the kernel is a distributed one-hot materialization: B*T = 2048 rows of
V = 32000 each, data-parallel over rows across 8 cores (256 rows/core).
The values are exactly 0/1, so the device materializes the tensor as one
byte per element (8.19 MB/core instead of 32.77 MB in f32) and the host
upcasts to f32 on gather; HBM write traffic, the sole bottleneck
(~358 GB/s per core), drops 4x.

Byte pairs are packed into int16 elements so the DVE compare runs in the
packed 2-byte modes: at most one of two adjacent columns holds a 1, so
  pair[j] = (j == label>>1) * (label even ? 1 : 256)
emits little-endian bytes (lo, hi) = (onehot[2j], onehot[2j+1]) in a single
tensor_scalar(is_equal, mult) per tile, with per-partition [128,1] scalar
operands.  Per core: 256 rows as [NB=2, 128p] x 16000 int16 pairs; iota,
label>>1 minus chunk base, and the even/odd scale are staged as int16
inputs.  Early chunks are narrow so the first compares only wait on a small
prefix of the iota load.
"""

import os
import sys

for _p in ("/opt/trn_rl_repo", "/root/.axon_site/_ro/trn_rl_repo"):
    if os.path.isdir(_p) and _p not in sys.path:
        sys.path.append(_p)

import numpy as np

B, T, V = 32, 64, 32000
N_CORES = 8
ROWS = B * T                 # 2048 one-hot rows total
RPC = ROWS // N_CORES        # 256 rows per core
NB = RPC // 128              # 2 batches of 128 partitions
VH = V // 2                  # 16000 int16 pairs per row
# Column-chunk widths in pair units.  Narrow leading chunks let the first
# compares start after a ~0.3us iota prefix load instead of the full 4KB.
CHUNKS_H = [500, 500, 1000] + [2000] * 7
assert sum(CHUNKS_H) == VH
NCHUNK = len(CHUNKS_H)
MAXH = max(CHUNKS_H)
IOTA_PIECES = [(0, 500), (500, 1000), (1000, 2000)]

_cache = {}


def _build_nc():
    import concourse.bacc as bacc
    import concourse.mybir as mybir
    from concourse.tile import TileContext

    nc = bacc.Bacc()
    iota_d = nc.dram_tensor("iota", [128, MAXH], mybir.dt.int16,
                            kind="ExternalInput")
    labm_d = nc.dram_tensor("labm", [128, NB * NCHUNK], mybir.dt.float32,
                            kind="ExternalInput")
    scl_d = nc.dram_tensor("scl", [128, NB], mybir.dt.float32,
                           kind="ExternalInput")
    out_d = nc.dram_tensor("out", [NB, 128, VH], mybir.dt.int16,
                           kind="ExternalOutput")

    with TileContext(nc) as tc:
        with tc.tile_pool(name="const", bufs=1) as cpool, \
             tc.tile_pool(name="work", bufs=6) as wpool:
            labm = cpool.tile([128, NB * NCHUNK], mybir.dt.float32, tag="labm")
            scl = cpool.tile([128, NB], mybir.dt.float32, tag="scl")
            iota = cpool.tile([128, MAXH], mybir.dt.int16, tag="iota")
            nc.sync.dma_start(out=labm[:, :], in_=labm_d[:, :])
            nc.sync.dma_start(out=scl[:, :], in_=scl_d[:, :])
            for (s, e) in IOTA_PIECES:
                nc.scalar.dma_start(out=iota[:, s:e], in_=iota_d[:, s:e])
            dma_engines = [nc.sync, nc.scalar]
            col = 0
            k = 0
            for ci, w in enumerate(CHUNKS_H):
                for b in range(NB):
                    o = wpool.tile([128, MAXH], mybir.dt.int16, tag="o")
                    # o = (iota == (label>>1) - chunk_base) * (1 or 256)
                    nc.vector.tensor_scalar(
                        out=o[:, :w], in0=iota[:, :w],
                        scalar1=labm[:, b * NCHUNK + ci:b * NCHUNK + ci + 1],
                        scalar2=scl[:, b:b + 1],
                        op0=mybir.AluOpType.is_equal,
                        op1=mybir.AluOpType.mult)
                    dma_engines[k % 2].dma_start(
                        out=out_d[b, :, col:col + w], in_=o[:, :w])
                    k += 1
                col += w
    nc.finalize()
    return nc


def kernel(**inputs):
    from concourse.bass_utils import run_bass_kernel_spmd

    if "nc" not in _cache:
        _cache["nc"] = _build_nc()
    nc = _cache["nc"]

    lab = np.asarray(inputs["labels"]).reshape(-1).astype(np.int64)
    bases = np.cumsum([0] + CHUNKS_H[:-1]).astype(np.int64)  # [NCHUNK]
    iota = np.tile(np.arange(MAXH, dtype=np.int16), (128, 1))
    in_maps = []
    for i in range(N_CORES):
        shard = lab[i * RPC:(i + 1) * RPC].reshape(NB, 128)   # [b, p]
        half = shard >> 1                                     # label // 2
        # labm[p, b*NCHUNK + c] = (label >> 1) - chunk_base[c]
        labm = (half[:, :, None] - bases[None, None, :])      # [b, p, c]
        labm = labm.transpose(1, 0, 2).reshape(128, NB * NCHUNK)
        scl = np.where(shard & 1, 256, 1).T                   # [p, b]
        in_maps.append({
            "iota": iota,
            "labm": np.ascontiguousarray(labm.astype(np.float32)),
            "scl": np.ascontiguousarray(scl.astype(np.float32)),
        })

    trace = bool(int(os.environ.get("BASS_KERNEL_TRACE", "0")))
    res = run_bass_kernel_spmd(nc, in_maps, list(range(N_CORES)), trace=trace)
    _cache["last_res"] = res

    outs = [np.asarray(res.results[i]["out"]).reshape(NB * 128, VH)
            for i in range(N_CORES)]
    packed = np.concatenate(outs, axis=0)                     # [ROWS, VH] i16
    return packed.view(np.uint8).astype(np.float32).reshape(B, T, V)
